# revision 21
# baseline (speedup 1.0000x reference)
"""GAT layer kernel for Trainium2, 8-core data-parallel over batch.

Math (per batch b, head h):
    h = x @ W                              [N, H*HD]
    s_n = <h[n, h*HD:(h+1)*HD], a_src[h]>  t_n likewise with a_dst
    A[j, i] = exp(leakyrelu(s_i + t_j, 0.2))
    out[i]  = (sum_j A[j, i] * h_j) / (sum_j A[j, i])

Key identity: softmax columns are scale-invariant, so drop the e^{s_i}
column factor entirely:
    A'[j, i] = A[j, i] * e^{-s_i} = max(e^{t_j}, e^{0.2 t_j} * u_i),
    u_i = e^{-0.8 s_i}
Both numerator and Z pick up the same e^{-s_i}, which cancels in the
division. Each [128, N] attention tile is then ONE vector tensor_scalar
op (in0 = broadcast u row, two per-partition scalar columns e^{t_j} /
e^{0.2 t_j}, ops mult+max).

Per core (= one batch element):
  - xT and [W | Wa] ship as fp16 (host precomputes Wa = W @ a_ext);
    xT halves lead their queues.
  - PE warm-up: the HAM clock gate defaults to 1.2 GHz and lifts to
    2.4 GHz only after a fully-busy free-running ~3.4us activity
    window. Eleven back-to-back full-K junk matmuls during the initial
    xT DMA wait, chaining bubble-free into the projection stream
    (verified: LDWEIGHTS are pulled ahead, no gaps), flip the gate
    mid-ramp instead of mid-main-loop. Five more junk matmuls are
    dep-gated into the ramp's known PE idle slots (behind uexp c0 and
    the u copy-outs) to keep the MID window from re-throttling; a
    ~0.8us TS-wait hole before the first main matmul remains and can
    still re-throttle on some runs (~1.5us penalty, run-dependent).
  - st rows = Wa_src^T @ xT per column half; u = Exp(-0.8 s) on ACT.
  - per j-tile the projection runs as TWO matmuls off one weight load:
    h_node (128 cols -> hn PSUM) and the a_dst t-columns (4 cols -> tt
    PSUM). Separate destination tiles matter: the dependency tracker
    serializes cross-engine accesses to the same tile, so with a
    combined tile the DVE weight carves and the ACT e^t exps would
    execute in series instead of in parallel.
  - u broadcast [1,N] -> [128,N]: head 0 via a K=1 ones-column outer
    product on the PE into per-half PSUM tiles (c0 copy-out on DVE, c1
    on ACT -- again separate tiles so the copies don't serialize);
    heads 1-3 via DRAM round-trip broadcast DMAs per column half on
    the sync/gpsimd queues only (a broadcast descriptor on the scalar
    queue stalls ACT compute behind the DRAM write's ~1.6us latency).
  - weight-tile carves (hn PSUM -> bf16 SBUF) on the then-idle DVE:
    the preamble PSUM pool releases only when ALL its readers are done
    (pool granularity) and the main loop's first accumulator banks
    gate on that release, so the carves/exps must finish early.
  - main loop per (head, j-tile): one tensor_scalar -> A' tile (bf16,
    full [128, N] -- per-op overhead makes half tiles ~80% costlier
    per element), two 512-col matmuls accumulate [h_node | ones]^T @
    A' into PSUM [33, N] (row 32 = Z). PE ~216 ns/512 cols vs DVE
    ~456 ns per A' tile -- closely matched.
  - per-head epilogue on ACT: Z rows -> zq (f32, partitions 0/32/64/
    96; dead lanes memset to 1.0 so the tail reciprocal stays
    finite), numerator rows -> num4 (head 3's split per half).
  - tail per column half: rz = 1/Z via ONE custom-DVE op
    (reciprocal_approx_fast, ~18 correct bits; ACT Reciprocal is gated
    off in bass, ln+exp thrashes activation-table loads, the 4-op
    Newton chain is 3x slower), DVE cast to bf16 (an fp32 indicator
    matmul runs LOW_HIGH double-pass at ~5x bf16 cost), K=97 bf16
    indicator matmul broadcasts rz to all 128 (h,d) rows, one fused
    num * rzb multiply, then the output ships per QUARTER on separate
    queues (the fixed ~1us queue latency dominates a 128KB transfer,
    so two quarter-DMAs in flight beat one half-DMA).
  - host transposes out^T back to node-major (bf16 -> f32) when
    unsharding.
"""

import numpy as np

B, N, IN_F, OUT_F, H = 8, 1024, 128, 128, 4
HD = OUT_F // H  # 32
NEG = 0.2
N_CORES = 8
NT = N // 128  # 8 node tiles

A_DT = "bfloat16"  # dtype of the attention tiles + matmul weights
N_WARM = 11  # full-K junk matmuls (512 cols each) to lift the HAM clock gate

_CACHE = {}


def _build_nc():
    import concourse.bacc as bacc
    import concourse.tile as tile
    from concourse import mybir

    f32 = mybir.dt.float32
    f16 = mybir.dt.float16
    adt = getattr(mybir.dt, A_DT)
    AF = mybir.ActivationFunctionType
    ALU = mybir.AluOpType

    nc = bacc.Bacc("TRN2", target_bir_lowering=False, debug=False,
                   num_devices=N_CORES)

    xT = nc.declare_dram_parameter("xT", [IN_F, N], f16, isOutput=False)
    WWa_d = nc.declare_dram_parameter("WWa", [IN_F, OUT_F + 2 * H], f16,
                                      isOutput=False)
    ind97_d = nc.declare_dram_parameter("ind97", [3 * HD + 1, OUT_F], adt,
                                        isOutput=False)
    outT = nc.declare_dram_parameter("outT", [OUT_F, N], adt, isOutput=True)

    u_dram = nc.dram_tensor("u_scratch", [H, N], adt)

    with tile.TileContext(nc) as tc:
      with (
        tc.tile_pool(name="const", bufs=1) as cpool,
        tc.tile_pool(name="atile", bufs=12) as apool,
        tc.tile_pool(name="tail", bufs=1) as tpool,
        tc.tile_pool(name="ps_ub", bufs=1, space="PSUM") as psub,
      ):
        # warm-up / ones operands come from the framework's own const
        # tiles via stride-0 broadcast APs: those memsets run ~1us
        # before any kernel-emitted gpsimd op can, so the junk matmuls
        # start during the framework preamble itself
        one_c = nc.const_aps.aps[(adt, 1.0)]
        w128 = one_c.to_broadcast([128, 128])
        wsrc = one_c.to_broadcast([128, 512])
        ones1 = one_c[0:1, :].to_broadcast([1, 128])

        # ---- input loads: xT halves lead their queues; ind97's
        # descriptor is wait-free so it can't stall later ACT compute
        xT_sb = cpool.tile([IN_F, N], f16, tag="xT")
        WW16 = cpool.tile([IN_F, OUT_F + 2 * H], f16, tag="WW16")
        nc.sync.dma_start(out=xT_sb[:, 0:512], in_=xT[:, 0:512])
        nc.sync.dma_start(out=WW16, in_=WWa_d[:])
        nc.scalar.dma_start(out=xT_sb[:, 512:N], in_=xT[:, 512:N])
        ind97 = cpool.tile([3 * HD + 1, OUT_F], adt, tag="ind97")
        nc.scalar.dma_start(out=ind97, in_=ind97_d[:])

        War16 = WW16[:, OUT_F:OUT_F + H]        # a_src columns
        Wat16 = WW16[:, OUT_F + H:OUT_F + 2 * H]  # a_dst columns

        u_rows = cpool.tile([H, N], adt, tag="u_rows")
        # per-half u outer-product targets (separate tiles: the c0/c1
        # copy-outs run on different engines and must not serialize)
        ub_ps = [psub.tile([128, 512], f32, tag=f"ubps{c}",
                           name=f"ubps{c}") for c in range(2)]
        u_b0 = cpool.tile([128, N], adt, tag="ub0")
        u_b = [u_b0]
        for h in range(1, H):
            u_b.append(cpool.tile([128, N], adt, name=f"ub{h}",
                                  tag=f"ub{h}"))

        etc = cpool.tile([128, H * NT], f32, tag="etc")
        etc02 = cpool.tile([128, H * NT], f32, tag="etc02")
        etc_v = etc[:].rearrange("p (h jt) -> p jt h", jt=NT)
        etc02_v = etc02[:].rearrange("p (h jt) -> p jt h", jt=NT)
        wt_all = cpool.tile([128, NT * 33 * H], adt, tag="wt")
        wt_v = wt_all[:].rearrange("p (jt h c) -> p jt h c", h=H, c=33)
        nc.gpsimd.memset(wt_v[:, :, :, 32:33], 1.0)

        # tail tensors: zq rows land on partitions 0/32/64/96; dead
        # lanes memset to 1.0 so reciprocal_approx_fast (undefined at
        # 0) stays finite under the rzb matmul's zero weights
        NP = 3 * HD + 1
        num4 = tpool.tile([128, N], f32, tag="num4")
        zq = tpool.tile([NP, N], f32, tag="zq")
        nc.gpsimd.memset(zq, 1.0)
        rz = tpool.tile([NP, N], f32, tag="rz")
        rzb16 = tpool.tile([NP, N], adt, tag="rzb16")

        with tc.tile_pool(name="ps_pre", bufs=1, space="PSUM") as pspre:
            st_ps = pspre.tile([H, N], f32, tag="st")
            hn_ps = pspre.tile([128, NT * OUT_F], f32, tag="hn")
            tt_ps = pspre.tile([128, NT * H], f32, tag="tt")
            hn_v = hn_ps[:].rearrange("p (jt h d) -> p jt h d", h=H, d=HD)
            tt_v = tt_ps[:].rearrange("p (jt h) -> p jt h", h=H)

            # ---- HAM warm-up: contiguous full-K junk matmuls ----
            for _ in range(N_WARM):
                nc.tensor.matmul(ub_ps[0][:, :], w128, wsrc,
                                 start=True, stop=True)

            # ---- ramp, fully 512-column-chunked ----
            def half_chain(c):
                cs = slice(512 * c, 512 * (c + 1))
                nc.tensor.matmul(st_ps[:, cs], War16, xT_sb[:, cs],
                                 start=True, stop=True)
                nc.scalar.activation(out=u_rows[:, cs], in_=st_ps[:, cs],
                                     func=AF.Exp, scale=-0.8)
                for jt in range(4 * c, 4 * (c + 1)):
                    # two matmuls off one weight load: h_node block and
                    # the t columns, into SEPARATE tiles
                    nc.tensor.matmul(
                        hn_ps[:, OUT_F * jt:OUT_F * (jt + 1)],
                        xT_sb[:, 128 * jt:128 * (jt + 1)],
                        WW16[:, 0:OUT_F], start=True, stop=True)
                    nc.tensor.matmul(
                        tt_ps[:, H * jt:H * (jt + 1)],
                        xT_sb[:, 128 * jt:128 * (jt + 1)],
                        Wat16, start=True, stop=True)
                # heads 1-3 u broadcast via DRAM round trip, per column
                # half, on the sync/gpsimd queues only
                nc.sync.dma_start(out=u_dram[:, cs], in_=u_rows[:, cs])
                nc.sync.dma_start(
                    out=u_b[1][:, cs],
                    in_=u_dram[1:2, cs].to_broadcast([128, 512]))
                for h in (2, 3):
                    nc.gpsimd.dma_start(
                        out=u_b[h][:, cs],
                        in_=u_dram[h:h + 1, cs].to_broadcast([128, 512]))

            half_chain(0)
            # weight carves on DVE (e^t exps on ACT read a disjoint
            # tile, so they run concurrently)
            nc.vector.tensor_copy(out=wt_v[:, 0:2, :, 0:32],
                                  in_=hn_v[:, 0:2])
            nc.vector.tensor_copy(out=wt_v[:, 2:4, :, 0:32],
                                  in_=hn_v[:, 2:4])
            half_chain(1)
            nc.vector.tensor_copy(out=wt_v[:, 4:, :, 0:32],
                                  in_=hn_v[:, 4:])
            # gap filler: no deps, so it executes right when the PE
            # reaches it -- covering the idle slot where the first
            # outer product waits on uexp c0 (the HAM MID window
            # re-throttles the clock even at ~40% idle)
            nc.tensor.matmul(ub_ps[0][:, :], w128, wsrc,
                             start=True, stop=True)
            # head-0 u broadcast: K=1 outer product per half into the
            # per-half PSUM tiles; c0 copy-out on DVE, c1 on ACT
            nc.tensor.matmul(ub_ps[0][:, :], ones1, u_rows[0:1, 0:512],
                             start=True, stop=True)
            nc.tensor.matmul(ub_ps[1][:, :], ones1, u_rows[0:1, 512:N],
                             start=True, stop=True)
            nc.vector.tensor_copy(out=u_b0[:, 0:512], in_=ub_ps[0][:, :])
            nc.scalar.copy(out=u_b0[:, 512:N], in_=ub_ps[1][:, :])
            # final fillers, gated on the c1 copy-out's OUTPUT: they
            # start the moment the last ACT ramp op lands and cover the
            # first A' tile's DVE time, shrinking the PE idle hole that
            # re-throttles the HAM clock gate
            for _ in range(3):
                nc.tensor.matmul(ub_ps[1][:, 0:256], w128,
                                 u_b0[:, 512:768], start=True, stop=True)
            # e^t columns batched as two whole-range ACT ops (per-op
            # overhead dominates the tiny 32-element exps); emitted
            # after the ub copy so the pool-release readers finish as
            # one short ACT burst
            nc.scalar.activation(out=etc_v[:, :], in_=tt_v[:, :],
                                 func=AF.Exp)
            nc.scalar.activation(out=etc02_v[:, :], in_=tt_v[:, :],
                                 func=AF.Exp, scale=NEG)
            # gap fillers: the first pair is dep-gated behind the c0
            # copy-out (WAR on ub_ps[0]), the second pair behind the c1
            # copy-out, so they self-schedule across the whole
            # ramp->main-loop PE idle window and keep the HAM MID
            # window from re-throttling the clock
            for _ in range(2):
                nc.tensor.matmul(ub_ps[0][:, :], w128, wsrc,
                                 start=True, stop=True)
            for _ in range(2):
                nc.tensor.matmul(ub_ps[1][:, :], w128, wsrc,
                                 start=True, stop=True)
            wts = [wt_all[:, 132 * jt:132 * (jt + 1)] for jt in range(NT)]

        # ---- main loop: one tensor_scalar + two matmuls per (h, jt);
        # bufs=3 so head 3 reuses head 0's banks (released mid-loop by
        # head 0's epilogue copies) ----
        with tc.tile_pool(name="ps_main", bufs=3, space="PSUM") as psmain:
            for h in range(H):
                oh = psmain.tile([33, N], f32, tag="oh")
                for jt in range(NT):
                    idx = h * NT + jt
                    a_t = apool.tile([128, N], adt, tag="at")
                    nc.vector.tensor_scalar(
                        out=a_t, in0=u_b[h],
                        scalar1=etc02[:, idx:idx + 1],
                        scalar2=etc[:, idx:idx + 1],
                        op0=ALU.mult, op1=ALU.max)
                    for c in range(2):
                        nc.tensor.matmul(
                            oh[:, 512 * c:512 * (c + 1)],
                            wts[jt][:, 33 * h:33 * (h + 1)],
                            a_t[:, 512 * c:512 * (c + 1)],
                            start=(jt == 0), stop=(jt == NT - 1))
                # per-head epilogue on ACT, pipelined with later heads'
                # bulk work
                for c in range(2):
                    cs = slice(512 * c, 512 * (c + 1))
                    nc.scalar.copy(out=zq[HD * h:HD * h + 1, cs],
                                   in_=oh[32:33, cs])
                if h == 3:
                    for c in range(2):
                        cs = slice(512 * c, 512 * (c + 1))
                        nc.scalar.copy(out=num4[HD * h:HD * (h + 1), cs],
                                       in_=oh[0:32, cs])
                else:
                    nc.scalar.copy(out=num4[HD * h:HD * (h + 1), :],
                                   in_=oh[0:32, :])

        # ---- tail per column half: rz = 1/Z (one custom-DVE op),
        # bf16 cast, K=97 bf16 indicator matmul, fused num * rzb
        # multiply, quarter-split bf16 DMA out ----
        with tc.tile_pool(name="ps_norm", bufs=2, space="PSUM") as psnorm:
            rzbs = []
            for c in range(2):
                cs = slice(512 * c, 512 * (c + 1))
                nc.vector.reciprocal_approx_fast(out=rz[:, cs],
                                                 in_=zq[:, cs])
                nc.vector.tensor_copy(out=rzb16[:, cs], in_=rz[:, cs])
                rzb = psnorm.tile([128, 512], f32, tag=f"rzb{c}")
                nc.tensor.matmul(rzb[:, :], ind97, rzb16[:, cs],
                                 start=True, stop=True)
                rzbs.append(rzb)
            qeng = [nc.sync, nc.gpsimd, nc.scalar, nc.gpsimd]
            for c in range(2):
                cs = slice(512 * c, 512 * (c + 1))
                o_sb = tpool.tile([128, 512], adt, tag=f"osb{c}")
                # quarter-granular multiply + DMA: each 64KB quarter
                # ships as soon as its multiply lands, hiding the
                # ~0.6us descriptor+queue latency behind the next
                # quarter's compute
                for q in range(2):
                    qs = slice(512 * c + 256 * q, 512 * c + 256 * (q + 1))
                    nc.vector.scalar_tensor_tensor(
                        out=o_sb[:, 256 * q:256 * (q + 1)],
                        in0=num4[:, qs], scalar=1.0,
                        in1=rzbs[c][:, 256 * q:256 * (q + 1)],
                        op0=ALU.mult, op1=ALU.mult)
                    qeng[2 * c + q].dma_start(out=outT[:, qs],
                                              in_=o_sb[:, 256 * q:
                                                       256 * (q + 1)])

    nc.compile()
    return nc


def _get_nc():
    if "nc" not in _CACHE:
        _CACHE["nc"] = _build_nc()
    return _CACHE["nc"]


def make_in_maps(x, W, a_src, a_dst):
    a_ext = np.zeros((OUT_F, 2 * H), np.float32)
    for h in range(H):
        a_ext[h * HD:(h + 1) * HD, h] = a_src[h]
        a_ext[h * HD:(h + 1) * HD, H + h] = a_dst[h]
    Wa = W @ a_ext
    # ind97[k, p] = 1 iff k == 32*(p//32) (Z-row broadcast), pre-cast
    # to bf16 so the device needs no conversion
    import ml_dtypes
    ind97 = np.zeros((3 * HD + 1, OUT_F), ml_dtypes.bfloat16)
    for h in range(H):
        ind97[HD * h, HD * h:HD * (h + 1)] = 1.0
    return [
        {"xT": np.ascontiguousarray(x[c].T).astype(np.float16),
         "WWa": np.concatenate([W, Wa], axis=1).astype(np.float16),
         "ind97": ind97}
        for c in range(N_CORES)
    ]


def kernel(x, W, a_src, a_dst):
    from concourse.bass_utils import run_bass_kernel_spmd

    x = np.asarray(x, dtype=np.float32)
    W = np.asarray(W, dtype=np.float32)
    a_src = np.asarray(a_src, dtype=np.float32)
    a_dst = np.asarray(a_dst, dtype=np.float32)

    nc = _get_nc()
    in_maps = make_in_maps(x, W, a_src, a_dst)
    res = run_bass_kernel_spmd(nc, in_maps, core_ids=list(range(N_CORES)))
    out = np.stack([np.asarray(res.results[c]["outT"]).astype(np.float32).T
                    for c in range(N_CORES)], axis=0)
    return np.ascontiguousarray(out, dtype=np.float32)


# revision 23
# speedup vs baseline: 1.0389x; 1.0389x over previous
"""GAT layer kernel for Trainium2, 8-core data-parallel over batch.

Math (per batch b, head h):
    h = x @ W                              [N, H*HD]
    s_n = <h[n, h*HD:(h+1)*HD], a_src[h]>  t_n likewise with a_dst
    A[j, i] = exp(leakyrelu(s_i + t_j, 0.2))
    out[i]  = (sum_j A[j, i] * h_j) / (sum_j A[j, i])

Key identity: softmax columns are scale-invariant, so drop the e^{s_i}
column factor entirely:
    A'[j, i] = A[j, i] * e^{-s_i} = max(e^{t_j}, e^{0.2 t_j} * u_i),
    u_i = e^{-0.8 s_i}
Both numerator and Z pick up the same e^{-s_i}, which cancels in the
division. Each [128, N] attention tile is then ONE vector tensor_scalar
op (in0 = broadcast u row, two per-partition scalar columns e^{t_j} /
e^{0.2 t_j}, ops mult+max).

Per core (= one batch element):
  - xT and [W | Wa] ship as fp16 (host precomputes Wa = W @ a_ext);
    xT halves lead their queues.
  - PE warm-up: the HAM clock gate defaults to 1.2 GHz and lifts to
    2.4 GHz only after a fully-busy free-running ~3.4us activity
    window. Eleven back-to-back full-K junk matmuls during the initial
    xT DMA wait, chaining bubble-free into the projection stream
    (verified: LDWEIGHTS are pulled ahead, no gaps), flip the gate
    mid-ramp instead of mid-main-loop. Five more junk matmuls are
    dep-gated into the ramp's known PE idle slots (behind uexp c0 and
    the u copy-outs). A ~0.85us hole remains between the last ACT ramp
    op and the first A'-gated matmul; the MID window re-throttles on
    it most runs (~1.5us penalty) -- filler matmuls gated on the A'
    tile, the u copy-out, or its output all failed to cover it
    (measured; they share the same gating event as the real matmul).
  - st rows = Wa_src^T @ xT per column half; u = Exp(-0.8 s) on ACT.
  - per j-tile the projection runs as TWO matmuls off one weight load:
    h_node (128 cols -> hn PSUM) and the a_dst t-columns (4 cols -> tt
    PSUM). Separate destination tiles matter: the dependency tracker
    serializes cross-engine accesses to the same tile, so with a
    combined tile the DVE weight carves and the ACT e^t exps would
    execute in series instead of in parallel.
  - u broadcast [1,N] -> [128,N]: head 0 via a K=1 ones-column outer
    product on the PE into per-half PSUM tiles (c0 copy-out on DVE, c1
    on ACT -- again separate tiles so the copies don't serialize);
    heads 1-3 via DRAM round-trip broadcast DMAs per column half on
    the sync/gpsimd queues only (a broadcast descriptor on the scalar
    queue stalls ACT compute behind the DRAM write's ~1.6us latency).
  - weight-tile carves (hn PSUM -> bf16 SBUF) on the then-idle DVE:
    the preamble PSUM pool releases only when ALL its readers are done
    (pool granularity) and the main loop's first accumulator banks
    gate on that release, so the carves/exps must finish early.
  - main loop per (head, j-tile): one tensor_scalar -> A' tile (bf16,
    full [128, N] -- per-op overhead makes half tiles ~80% costlier
    per element), two 512-col matmuls accumulate [h_node | ones]^T @
    A' into PSUM [33, N] (row 32 = Z). PE ~216 ns/512 cols vs DVE
    ~456 ns per A' tile -- closely matched.
  - per-head epilogue on ACT: Z rows -> zq (f32, partitions 0/32/64/
    96; dead lanes memset to 1.0 so the tail reciprocal stays
    finite), numerator rows -> num4 (head 3's split per half).
  - tail per column half: rz = 1/Z via ONE custom-DVE op
    (reciprocal_approx_fast, ~18 correct bits; ACT Reciprocal is gated
    off in bass, ln+exp thrashes activation-table loads, the 4-op
    Newton chain is 3x slower), DVE cast to bf16 (an fp32 indicator
    matmul runs LOW_HIGH double-pass at ~5x bf16 cost), K=97 bf16
    indicator matmul broadcasts rz to all 128 (h,d) rows, one fused
    num * rzb multiply, then the output ships per QUARTER on separate
    queues (the fixed ~1us queue latency dominates a 128KB transfer,
    so two quarter-DMAs in flight beat one half-DMA).
  - host transposes out^T back to node-major (bf16 -> f32) when
    unsharding.
"""

import numpy as np

B, N, IN_F, OUT_F, H = 8, 1024, 128, 128, 4
HD = OUT_F // H  # 32
NEG = 0.2
N_CORES = 8
NT = N // 128  # 8 node tiles

A_DT = "bfloat16"  # dtype of the attention tiles + matmul weights
N_WARM = 11  # full-K junk matmuls (512 cols each) to lift the HAM clock gate

_CACHE = {}


def _build_nc():
    import concourse.bacc as bacc
    import concourse.tile as tile
    from concourse import mybir

    f32 = mybir.dt.float32
    f16 = mybir.dt.float16
    adt = getattr(mybir.dt, A_DT)
    AF = mybir.ActivationFunctionType
    ALU = mybir.AluOpType

    nc = bacc.Bacc("TRN2", target_bir_lowering=False, debug=False,
                   num_devices=N_CORES)

    xT = nc.declare_dram_parameter("xT", [IN_F, N], f16, isOutput=False)
    WWa_d = nc.declare_dram_parameter("WWa", [IN_F, OUT_F + 2 * H], f16,
                                      isOutput=False)
    ind97_d = nc.declare_dram_parameter("ind97", [3 * HD + 1, OUT_F], adt,
                                        isOutput=False)
    outT = nc.declare_dram_parameter("outT", [OUT_F, N], adt, isOutput=True)

    u_dram = nc.dram_tensor("u_scratch", [H, N], adt)

    with tile.TileContext(nc) as tc:
      with (
        tc.tile_pool(name="const", bufs=1) as cpool,
        tc.tile_pool(name="atile", bufs=12) as apool,
        tc.tile_pool(name="tail", bufs=1) as tpool,
        tc.tile_pool(name="ps_ub", bufs=1, space="PSUM") as psub,
      ):
        # warm-up / ones operands come from the framework's own const
        # tiles via stride-0 broadcast APs: those memsets run ~1us
        # before any kernel-emitted gpsimd op can, so the junk matmuls
        # start during the framework preamble itself
        one_c = nc.const_aps.aps[(adt, 1.0)]
        w128 = one_c.to_broadcast([128, 128])
        wsrc = one_c.to_broadcast([128, 512])
        ones1 = one_c[0:1, :].to_broadcast([1, 128])

        # ---- input loads: xT halves lead their queues; ind97's
        # descriptor is wait-free so it can't stall later ACT compute
        xT_sb = cpool.tile([IN_F, N], f16, tag="xT")
        WW16 = cpool.tile([IN_F, OUT_F + 2 * H], f16, tag="WW16")
        nc.sync.dma_start(out=xT_sb[:, 0:512], in_=xT[:, 0:512])
        nc.sync.dma_start(out=WW16, in_=WWa_d[:])
        nc.scalar.dma_start(out=xT_sb[:, 512:N], in_=xT[:, 512:N])
        ind97 = cpool.tile([3 * HD + 1, OUT_F], adt, tag="ind97")
        nc.scalar.dma_start(out=ind97, in_=ind97_d[:])

        War16 = WW16[:, OUT_F:OUT_F + H]        # a_src columns
        Wat16 = WW16[:, OUT_F + H:OUT_F + 2 * H]  # a_dst columns

        u_rows = cpool.tile([H, N], adt, tag="u_rows")
        # per-half u outer-product targets (separate tiles: the c0/c1
        # copy-outs run on different engines and must not serialize)
        ub_ps = [psub.tile([128, 512], f32, tag=f"ubps{c}",
                           name=f"ubps{c}") for c in range(2)]
        u_b0 = cpool.tile([128, N], adt, tag="ub0")
        u_b = [u_b0]
        for h in range(1, H):
            u_b.append(cpool.tile([128, N], adt, name=f"ub{h}",
                                  tag=f"ub{h}"))

        etc = cpool.tile([128, H * NT], f32, tag="etc")
        etc02 = cpool.tile([128, H * NT], f32, tag="etc02")
        etc_v = etc[:].rearrange("p (h jt) -> p jt h", jt=NT)
        etc02_v = etc02[:].rearrange("p (h jt) -> p jt h", jt=NT)
        wt_all = cpool.tile([128, NT * 33 * H], adt, tag="wt")
        wt_v = wt_all[:].rearrange("p (jt h c) -> p jt h c", h=H, c=33)
        nc.gpsimd.memset(wt_v[:, :, :, 32:33], 1.0)

        # tail tensors: zq rows land on partitions 0/32/64/96; dead
        # lanes memset to 1.0 so reciprocal_approx_fast (undefined at
        # 0) stays finite under the rzb matmul's zero weights
        NP = 3 * HD + 1
        num4 = tpool.tile([128, N], f32, tag="num4")
        zq = tpool.tile([NP, N], f32, tag="zq")
        nc.gpsimd.memset(zq, 1.0)
        rz = tpool.tile([NP, N], f32, tag="rz")
        rzb16 = tpool.tile([NP, N], adt, tag="rzb16")

        with tc.tile_pool(name="ps_pre", bufs=1, space="PSUM") as pspre:
            st_ps = pspre.tile([H, N], f32, tag="st")
            hn_ps = pspre.tile([128, NT * OUT_F], f32, tag="hn")
            tt_ps = pspre.tile([128, NT * H], f32, tag="tt")
            hn_v = hn_ps[:].rearrange("p (jt h d) -> p jt h d", h=H, d=HD)
            tt_v = tt_ps[:].rearrange("p (jt h) -> p jt h", h=H)

            # ---- HAM warm-up: contiguous full-K junk matmuls ----
            for _ in range(N_WARM):
                nc.tensor.matmul(ub_ps[0][:, :], w128, wsrc,
                                 start=True, stop=True)

            # ---- ramp, fully 512-column-chunked ----
            def half_chain(c):
                cs = slice(512 * c, 512 * (c + 1))
                nc.tensor.matmul(st_ps[:, cs], War16, xT_sb[:, cs],
                                 start=True, stop=True)
                nc.scalar.activation(out=u_rows[:, cs], in_=st_ps[:, cs],
                                     func=AF.Exp, scale=-0.8)
                for jt in range(4 * c, 4 * (c + 1)):
                    # two matmuls off one weight load: h_node block and
                    # the t columns, into SEPARATE tiles
                    nc.tensor.matmul(
                        hn_ps[:, OUT_F * jt:OUT_F * (jt + 1)],
                        xT_sb[:, 128 * jt:128 * (jt + 1)],
                        WW16[:, 0:OUT_F], start=True, stop=True)
                    nc.tensor.matmul(
                        tt_ps[:, H * jt:H * (jt + 1)],
                        xT_sb[:, 128 * jt:128 * (jt + 1)],
                        Wat16, start=True, stop=True)
                # heads 1-3 u broadcast via DRAM round trip, per column
                # half, on the sync/gpsimd queues only
                nc.sync.dma_start(out=u_dram[:, cs], in_=u_rows[:, cs])
                nc.sync.dma_start(
                    out=u_b[1][:, cs],
                    in_=u_dram[1:2, cs].to_broadcast([128, 512]))
                for h in (2, 3):
                    nc.gpsimd.dma_start(
                        out=u_b[h][:, cs],
                        in_=u_dram[h:h + 1, cs].to_broadcast([128, 512]))

            half_chain(0)
            # weight carves on DVE (e^t exps on ACT read a disjoint
            # tile, so they run concurrently)
            nc.vector.tensor_copy(out=wt_v[:, 0:2, :, 0:32],
                                  in_=hn_v[:, 0:2])
            nc.vector.tensor_copy(out=wt_v[:, 2:4, :, 0:32],
                                  in_=hn_v[:, 2:4])
            half_chain(1)
            nc.vector.tensor_copy(out=wt_v[:, 4:, :, 0:32],
                                  in_=hn_v[:, 4:])
            # gap filler: no deps, so it executes right when the PE
            # reaches it -- covering the idle slot where the first
            # outer product waits on uexp c0 (the HAM MID window
            # re-throttles the clock even at ~40% idle)
            nc.tensor.matmul(ub_ps[0][:, :], w128, wsrc,
                             start=True, stop=True)
            # head-0 u broadcast: K=1 outer product per half into the
            # per-half PSUM tiles; c0 copy-out on DVE, c1 on ACT
            nc.tensor.matmul(ub_ps[0][:, :], ones1, u_rows[0:1, 0:512],
                             start=True, stop=True)
            nc.tensor.matmul(ub_ps[1][:, :], ones1, u_rows[0:1, 512:N],
                             start=True, stop=True)
            nc.vector.tensor_copy(out=u_b0[:, 0:512], in_=ub_ps[0][:, :])
            nc.scalar.copy(out=u_b0[:, 512:N], in_=ub_ps[1][:, :])
            # e^t columns batched as two whole-range ACT ops (per-op
            # overhead dominates the tiny 32-element exps); emitted
            # after the ub copy so the pool-release readers finish as
            # one short ACT burst
            nc.scalar.activation(out=etc_v[:, :], in_=tt_v[:, :],
                                 func=AF.Exp)
            nc.scalar.activation(out=etc02_v[:, :], in_=tt_v[:, :],
                                 func=AF.Exp, scale=NEG)
            # gap fillers: the first pair is dep-gated behind the c0
            # copy-out (WAR on ub_ps[0]), the second pair behind the c1
            # copy-out, so they self-schedule across the whole
            # ramp->main-loop PE idle window and keep the HAM MID
            # window from re-throttling the clock
            for _ in range(2):
                nc.tensor.matmul(ub_ps[0][:, :], w128, wsrc,
                                 start=True, stop=True)
            for _ in range(2):
                nc.tensor.matmul(ub_ps[1][:, :], w128, wsrc,
                                 start=True, stop=True)
            wts = [wt_all[:, 132 * jt:132 * (jt + 1)] for jt in range(NT)]

        # ---- main loop: one tensor_scalar + two matmuls per (h, jt);
        # bufs=3 so head 3 reuses head 0's banks (released mid-loop by
        # head 0's epilogue copies) ----
        with tc.tile_pool(name="ps_main", bufs=3, space="PSUM") as psmain:
            for h in range(H):
                oh = psmain.tile([33, N], f32, tag="oh")
                for jt in range(NT):
                    idx = h * NT + jt
                    a_t = apool.tile([128, N], adt, tag="at")
                    nc.vector.tensor_scalar(
                        out=a_t, in0=u_b[h],
                        scalar1=etc02[:, idx:idx + 1],
                        scalar2=etc[:, idx:idx + 1],
                        op0=ALU.mult, op1=ALU.max)
                    for c in range(2):
                        nc.tensor.matmul(
                            oh[:, 512 * c:512 * (c + 1)],
                            wts[jt][:, 33 * h:33 * (h + 1)],
                            a_t[:, 512 * c:512 * (c + 1)],
                            start=(jt == 0), stop=(jt == NT - 1))
                # per-head epilogue on ACT, pipelined with later heads'
                # bulk work. Head 3's Z c0 copy runs on the then-idle
                # DVE instead: it leads the DVE FIFO straight into the
                # reciprocal chain, while ACT does the c1/num copies in
                # parallel -- removing one serial ACT hop from the tail
                for c in range(2):
                    cs = slice(512 * c, 512 * (c + 1))
                    if h == 3 and c == 0:
                        nc.vector.tensor_copy(out=zq[HD * h:HD * h + 1, cs],
                                              in_=oh[32:33, cs])
                    else:
                        nc.scalar.copy(out=zq[HD * h:HD * h + 1, cs],
                                       in_=oh[32:33, cs])
                if h == 3:
                    for c in range(2):
                        cs = slice(512 * c, 512 * (c + 1))
                        nc.scalar.copy(out=num4[HD * h:HD * (h + 1), cs],
                                       in_=oh[0:32, cs])
                else:
                    nc.scalar.copy(out=num4[HD * h:HD * (h + 1), :],
                                   in_=oh[0:32, :])

        # ---- tail per column half: rz = 1/Z (one custom-DVE op),
        # bf16 cast, K=97 bf16 indicator matmul, fused num * rzb
        # multiply, quarter-split bf16 DMA out ----
        with tc.tile_pool(name="ps_norm", bufs=2, space="PSUM") as psnorm:
            rzbs = []
            for c in range(2):
                cs = slice(512 * c, 512 * (c + 1))
                nc.vector.reciprocal_approx_fast(out=rz[:, cs],
                                                 in_=zq[:, cs])
                nc.vector.tensor_copy(out=rzb16[:, cs], in_=rz[:, cs])
                rzb = psnorm.tile([128, 512], f32, tag=f"rzb{c}")
                nc.tensor.matmul(rzb[:, :], ind97, rzb16[:, cs],
                                 start=True, stop=True)
                rzbs.append(rzb)
            qeng = [nc.sync, nc.gpsimd, nc.scalar, nc.gpsimd]
            for c in range(2):
                cs = slice(512 * c, 512 * (c + 1))
                o_sb = tpool.tile([128, 512], adt, tag=f"osb{c}")
                nc.vector.scalar_tensor_tensor(
                    out=o_sb, in0=num4[:, cs],
                    scalar=1.0, in1=rzbs[c], op0=ALU.mult, op1=ALU.mult)
                for q in range(2):
                    qs = slice(512 * c + 256 * q, 512 * c + 256 * (q + 1))
                    qeng[2 * c + q].dma_start(out=outT[:, qs],
                                              in_=o_sb[:, 256 * q:
                                                       256 * (q + 1)])

    nc.compile()
    return nc


def _get_nc():
    if "nc" not in _CACHE:
        _CACHE["nc"] = _build_nc()
    return _CACHE["nc"]


def make_in_maps(x, W, a_src, a_dst):
    a_ext = np.zeros((OUT_F, 2 * H), np.float32)
    for h in range(H):
        a_ext[h * HD:(h + 1) * HD, h] = a_src[h]
        a_ext[h * HD:(h + 1) * HD, H + h] = a_dst[h]
    Wa = W @ a_ext
    # ind97[k, p] = 1 iff k == 32*(p//32) (Z-row broadcast), pre-cast
    # to bf16 so the device needs no conversion
    import ml_dtypes
    ind97 = np.zeros((3 * HD + 1, OUT_F), ml_dtypes.bfloat16)
    for h in range(H):
        ind97[HD * h, HD * h:HD * (h + 1)] = 1.0
    return [
        {"xT": np.ascontiguousarray(x[c].T).astype(np.float16),
         "WWa": np.concatenate([W, Wa], axis=1).astype(np.float16),
         "ind97": ind97}
        for c in range(N_CORES)
    ]


def kernel(x, W, a_src, a_dst):
    from concourse.bass_utils import run_bass_kernel_spmd

    x = np.asarray(x, dtype=np.float32)
    W = np.asarray(W, dtype=np.float32)
    a_src = np.asarray(a_src, dtype=np.float32)
    a_dst = np.asarray(a_dst, dtype=np.float32)

    nc = _get_nc()
    in_maps = make_in_maps(x, W, a_src, a_dst)
    res = run_bass_kernel_spmd(nc, in_maps, core_ids=list(range(N_CORES)))
    out = np.stack([np.asarray(res.results[c]["outT"]).astype(np.float32).T
                    for c in range(N_CORES)], axis=0)
    return np.ascontiguousarray(out, dtype=np.float32)
